# revision 33
# baseline (speedup 1.0000x reference)
"""Cosformer attention (causal linear attention with cos reweighting) on 8
Trainium2 NeuronCores.

Sharding: n = bsz*heads = 16 sequences -> 2 per core. Core c handles batch-half
i = c//4 and head-pair p = c%4 (heads 2p, 2p+1). Fully data/head parallel; the
only cross-core interaction is the host-side sum of output-projection partials.

Per-core kernel (L=1024 tokens, d=64 per head, pair feature dim P=128):
  1. Feat-major projections Q^T/K^T per head in TF32 (duplicated-W trick:
     weight slice [Wh.T | Wh.T] (512x128) so rows 0:64 / 64:128 both hold the
     head's features) -> relu(+bias) -> * [sin;cos] row table -> bf16 q_^T,k_^T.
     V^T projected once per pair.
  2. Chunked causal linear attention (bf16 matmuls, fp32 PSUM), chunk=128:
       B   = masked A^T (upper-tri j<=i)
       qkv = B.T @ V~  +  q^T.T @ S      (V~ = [V|1]; col 64 = denominator)
       S  += K_tok.T @ V~ in a persistent PSUM bank (fp32, no drift)
       attn = qkv[:,0:64] * 1/max(denom,eps)   (per-partition scalars)
  3. attn pair chunk -> PE transpose -> TF32 out-proj partial -> DRAM.
Host sums 4 partials per batch-half, adds bo, reinterleaves rows.

Inputs per core are packed into three DMA images (xt + 2 constant packs) to
minimize DMA trigger count; loads are split across the two HWDGE rings
(sync/SP and scalar/ACT); output partials go out via gpsimd SWDGE.
"""

import os
import sys

import numpy as np

for _p in ("/opt/trn_rl_repo", "/root/.axon_site/_ro/trn_rl_repo"):
    if os.path.isdir(_p) and _p not in sys.path:
        sys.path.insert(0, _p)

N_HEAD = 8
E = 512
L = 1024  # sequence length per batch-half
BSZ = 2
D = 64  # head dim
P = 128  # partition/chunk/pair-feature size
NCHUNK = L // P
EPS = 1e-6
N_CORES = 8
TH = 512  # token-half width for projections (f32r moving max)

# pack layouts (fp32 columns)
# crit1: [biases(4: bq_a,bq_b,bk_a,bk_b) | wq_a (512)] -- small, lands first
_C1_BIAS = 0
_C1_WQA = 4
_C1_COLS = 516
# crit2: [scb f32 (1024) | wq_b | wk_a | wk_b]
_C2_SCB = 0
_C2_W = 1024
_C2_COLS = 2560
# rest: [wv (512) | wo (512) | mask (128) | ident bf16 (64)]
_R_WV = 0
_R_WO = 512
_R_MASK = 1024
_R_IDENT = 1152
_R_COLS = 1216

_CACHE = {}


def _build_bass():
    import concourse.bass as bass
    import concourse.tile as tile
    from concourse import bacc, mybir
    from contextlib import ExitStack

    f32 = mybir.dt.float32
    f32r = mybir.dt.float32r
    bf16 = mybir.dt.bfloat16
    AF = mybir.ActivationFunctionType

    nc = bacc.Bacc("TRN2", target_bir_lowering=False, debug=False)

    xt_d = nc.dram_tensor("xt", [E, L], f32r, kind="ExternalInput")
    c1_d = nc.dram_tensor("c1", [P, _C1_COLS], f32r, kind="ExternalInput")
    c2_d = nc.dram_tensor("c2", [P, _C2_COLS], f32r, kind="ExternalInput")
    rp_d = nc.dram_tensor("rp", [P, _R_COLS], f32r, kind="ExternalInput")
    out_d = nc.dram_tensor("out", [L, E], f32, kind="ExternalOutput")

    with tile.TileContext(nc) as tc:
        with ExitStack() as ctx:
            ep = ctx.enter_context
            cpool = ep(tc.tile_pool(name="const", bufs=1))
            seqp = ep(tc.tile_pool(name="seq", bufs=1))
            ktokp = ep(tc.tile_pool(name="ktok", bufs=8))
            vtp = ep(tc.tile_pool(name="vt", bufs=10))
            bp = ep(tc.tile_pool(name="bsb", bufs=4))
            sp = ep(tc.tile_pool(name="state", bufs=4))
            app = ep(tc.tile_pool(name="apair", bufs=2))
            atp = ep(tc.tile_pool(name="attnT", bufs=2))
            outp = ep(tc.tile_pool(name="outsb", bufs=2))
            rp = ep(tc.tile_pool(name="rcol", bufs=4))
            big_ps = ep(tc.tile_pool(name="bigps", bufs=2, space="PSUM"))
            sq_ps = ep(tc.tile_pool(name="sqps", bufs=2, space="PSUM"))
            acc_ps = ep(tc.tile_pool(name="accps", bufs=2, space="PSUM"))
            s_ps = ep(tc.tile_pool(name="sps", bufs=1, space="PSUM"))

            # ---- loads: two HWDGE rings (sync + scalar) ----
            # sync: crit packs + xt token-half 1; scalar: xt token-half 0 + rest.
            c1_t = cpool.tile([P, _C1_COLS], f32r, name="c1_t")
            nc.sync.dma_start(c1_t[:], c1_d[:, :])
            c2_t = cpool.tile([P, _C2_COLS], f32r, name="c2_t")
            nc.sync.dma_start(c2_t[:], c2_d[:, :])
            xts = []
            for e in range(4):
                t = cpool.tile([P, L], f32r, name=f"xt{e}")
                nc.scalar.dma_start(t[:, 0:TH], xt_d[e * P : (e + 1) * P, 0:TH])
                nc.sync.dma_start(t[:, TH:L], xt_d[e * P : (e + 1) * P, TH:L])
                xts.append(t)
            rp_t = cpool.tile([P, _R_COLS], f32r, name="rp_t")
            nc.scalar.dma_start(rp_t[:], rp_d[:, :])

            wt = {}
            wt["wq_a"] = [
                c1_t[:, _C1_WQA + e * P : _C1_WQA + (e + 1) * P] for e in range(4)
            ]
            for wi, nm in enumerate(("wq_b", "wk_a", "wk_b")):
                wt[nm] = [
                    c2_t[:, _C2_W + wi * 512 + e * P : _C2_W + wi * 512 + (e + 1) * P]
                    for e in range(4)
                ]
            wt["wv"] = [
                rp_t[:, _R_WV + e * P : _R_WV + (e + 1) * P] for e in range(4)
            ]
            wo_t = rp_t[:, _R_WO : _R_WO + 512]
            scb_t = c2_t[:, _C2_SCB : _C2_SCB + 1024].bitcast(f32)
            mask_t = rp_t[:, _R_MASK : _R_MASK + 128].bitcast(f32)
            bt = {
                nm: c1_t[:, _C1_BIAS + i : _C1_BIAS + i + 1].bitcast(f32)
                for i, nm in enumerate(("bq_a", "bq_b", "bk_a", "bk_b"))
            }
            ident_t = rp_t[:, _R_IDENT : _R_IDENT + 64].bitcast(bf16)

            # ---- projections (feat-major, TF32 matmuls, bf16 outputs) ----
            q_seq = {h: seqp.tile([P, L], bf16, name=f"q_{h}") for h in "ab"}
            k_seq = {h: seqp.tile([P, L], bf16, name=f"k_{h}") for h in "ab"}
            v_seq = seqp.tile([P, L], bf16, name="v_pair")

            def project_half(seq, wname, bname, func, outname, do_scale, th):
                ps = big_ps.tile([P, TH], f32, tag="big", name=f"{outname}_ps{th}")
                for e in range(4):
                    nc.tensor.matmul(
                        ps[:],
                        wt[wname][e],
                        xts[e][:, th * TH : (th + 1) * TH],
                        start=(e == 0),
                        stop=(e == 3),
                    )
                sl = seq[:, th * TH : (th + 1) * TH]
                if bname is None:
                    nc.scalar.copy(sl, ps[:])
                else:
                    nc.scalar.activation(sl, ps[:], func, bias=bt[bname])
                if do_scale:
                    nc.vector.tensor_mul(sl, sl, scb_t[:, th * TH : (th + 1) * TH])

            def project_th(th):
                for h in "ab":
                    project_half(q_seq[h], f"wq_{h}", f"bq_{h}", AF.Relu, f"q_{h}", True, th)
                    project_half(k_seq[h], f"wk_{h}", f"bk_{h}", AF.Relu, f"k_{h}", True, th)
                project_half(v_seq, "wv", None, AF.Identity, "v_pair", False, th)

            # ---- attention ----
            # Per-head fp32 running state, each in its own persistent PSUM
            # bank (start=True zeroes a whole 2KB bank region, so groups can
            # never share a bank); bf16 snapshots feed the next chunk's inter.
            s_bank = {
                "a": s_ps.tile([P, D + 1], f32, name="s_bank_a"),
                "b": s_ps.tile([P, D + 1], f32, name="s_bank_b"),
            }
            S_prev = {"a": None, "b": None}
            vtiles = {}
            ktoks = {}

            def attn_chunk(c):
                cs = slice(c * P, (c + 1) * P)
                vt_ps = acc_ps.tile([P, P], bf16, tag="acc", name=f"vtps{c}")
                nc.tensor.matmul(vt_ps[:], v_seq[:, cs], ident_t, is_transpose=True)
                for j, h in enumerate("ab"):
                    vt = vtp.tile([P, D + 1], bf16, tag="vt", name=f"vt_{h}{c}")
                    nc.vector.tensor_copy(vt[:, 0:D], vt_ps[:, j * D : (j + 1) * D])
                    nc.gpsimd.memset(vt[:, D : D + 1], 1.0)
                    vtiles[h, c] = vt
                attn_pair = app.tile([P, P], bf16, tag="ap", name=f"ap{c}")
                for j, h in enumerate("ab"):
                    qc = q_seq[h][:, cs]
                    kc = k_seq[h][:, cs]
                    vt = vtiles[h, c]
                    # masked A^T
                    b_ps = sq_ps.tile([P, P], f32, tag="sq", name=f"bps_{h}{c}")
                    nc.tensor.matmul(b_ps[:], kc, qc, start=True, stop=True)
                    b_sb = bp.tile([P, P], bf16, tag="bsb", name=f"bsb_{h}{c}")
                    nc.vector.tensor_mul(b_sb[:], b_ps[:], mask_t)
                    # qkv = intra + inter
                    qkv = acc_ps.tile([P, D + 1], f32, tag="acc", name=f"qkv_{h}{c}")
                    nc.tensor.matmul(
                        qkv[:], b_sb[:], vt[:], start=True, stop=(c == 0)
                    )
                    if c > 0:
                        nc.tensor.matmul(
                            qkv[:], qc, S_prev[h][:], start=False, stop=True
                        )
                    # state update in persistent PSUM (skip on last chunk)
                    if c < NCHUNK - 1:
                        kt_ps = sq_ps.tile([P, P], bf16, tag="sq", name=f"ktps_{h}{c}")
                        nc.tensor.matmul(
                            kt_ps[:], kc, ident_t, is_transpose=True
                        )
                        ktok = ktokp.tile([P, P], bf16, tag="ktok", name=f"ktok_{h}{c}")
                        nc.vector.tensor_copy(ktok[:], kt_ps[:])
                        nc.tensor.matmul(
                            s_bank[h][:],
                            ktok[:],
                            vt[:],
                            start=(c == 0),
                            stop=(c == NCHUNK - 2),
                            skip_group_check=True,
                        )
                        s_new = sp.tile([P, D + 1], bf16, tag="S", name=f"S_{h}{c}")
                        nc.scalar.copy(s_new[:], s_bank[h][:])
                        S_prev[h] = s_new
                    # normalize
                    r_col = rp.tile([P, 2], f32, tag="r", name=f"r_{h}{c}")
                    nc.vector.tensor_scalar_max(r_col[:, 0:1], qkv[:, D : D + 1], EPS)
                    nc.vector.reciprocal(r_col[:, 1:2], r_col[:, 0:1])
                    nc.vector.tensor_scalar_mul(
                        attn_pair[:, j * D : (j + 1) * D], qkv[:, 0:D], r_col[:, 1:2]
                    )
                # out projection for this chunk (TF32)
                at_ps = acc_ps.tile([P, P], bf16, tag="acc", name=f"atps{c}")
                nc.tensor.matmul(at_ps[:], attn_pair[:], ident_t, is_transpose=True)
                at_sb = atp.tile([P, P], f32r, tag="at", name=f"at{c}")
                nc.scalar.copy(at_sb[:], at_ps[:])
                o_ps = big_ps.tile([P, E], f32, tag="big", name=f"ops{c}")
                nc.tensor.matmul(o_ps[:], at_sb[:], wo_t, start=True, stop=True)
                o_sb = outp.tile([P, E], f32, tag="osb", name=f"osb{c}")
                if c % 2 == 0:
                    nc.scalar.copy(o_sb[:], o_ps[:])
                else:
                    nc.vector.tensor_copy(o_sb[:], o_ps[:])
                nc.gpsimd.dma_start(out_d[cs, :], o_sb[:])

            project_th(0)
            project_th(1)
            for c in range(NCHUNK):
                attn_chunk(c)

    nc.compile()
    return nc


def _get_nc():
    if "nc" not in _CACHE:
        _CACHE["nc"] = _build_bass()
    return _CACHE["nc"]


def make_in_maps(query, Wq, bq, Wk, bk, Wv, bv, Wo, bo):
    import ml_dtypes

    f32 = np.float32
    query = np.asarray(query, f32)
    x3 = query.reshape(L, BSZ, E)  # faithful torch .view reshape
    idx = (np.pi / 2) * np.arange(1, L + 1, dtype=f32) / f32(L)
    sinv = np.sin(idx).astype(f32)
    cosv = np.cos(idx).astype(f32)

    Wq, Wk, Wv, Wo = (np.asarray(w, f32) for w in (Wq, Wk, Wv, Wo))
    bq, bk, bv = (np.asarray(b, f32) for b in (bq, bk, bv))

    def wslice_dup(W, h):
        """(128, 512): [Wh.T | Wh.T] dup cols laid out as 4 e-tiles of 128."""
        w = W[D * h : D * (h + 1), :].T  # (512, 64)
        wd = np.concatenate([w, w], axis=1)  # (512, 128)
        return np.hstack([wd[e * P : (e + 1) * P, :] for e in range(4)])

    def bdup(b, h):
        bb = b[D * h : D * (h + 1)]
        return np.concatenate([bb, bb]).astype(f32)

    ident_f32 = np.ascontiguousarray(np.eye(P, dtype=ml_dtypes.bfloat16)).view(f32)
    scb = np.empty((P, L), f32)
    scb[0:D] = sinv[None, :]
    scb[D:P] = cosv[None, :]
    mask = np.triu(np.ones((P, P), f32))

    in_maps = []
    for c in range(N_CORES):
        i, p = divmod(c, 4)
        hA, hB = 2 * p, 2 * p + 1

        biases = np.stack(
            [bdup(bq, hA), bdup(bq, hB), bdup(bk, hA), bdup(bk, hB)], axis=1
        ).astype(f32)  # (128, 4)
        c1 = np.hstack([biases, wslice_dup(Wq, hA)])
        assert c1.shape == (P, _C1_COLS), c1.shape
        c2 = np.hstack(
            [scb, wslice_dup(Wq, hB), wslice_dup(Wk, hA), wslice_dup(Wk, hB)]
        )
        assert c2.shape == (P, _C2_COLS), c2.shape

        wv_p = Wv[P * p : P * (p + 1), :].T  # (512, 128)
        wv_pack = np.hstack([wv_p[e * P : (e + 1) * P, :] for e in range(4)])
        wo_pack = Wo[:, P * p : P * (p + 1)].T  # (128, 512)
        rest = np.hstack([wv_pack, wo_pack, mask, ident_f32])
        assert rest.shape == (P, _R_COLS), rest.shape

        in_maps.append(
            dict(
                xt=np.ascontiguousarray(x3[:, i, :].T),
                c1=np.ascontiguousarray(c1),
                c2=np.ascontiguousarray(c2),
                rp=np.ascontiguousarray(rest),
            )
        )
    return in_maps


def assemble(partials, bo, bv, Wo):
    out_flat = np.zeros((BSZ * L, E), np.float32)
    out_flat[0::2] = partials[0] + partials[1] + partials[2] + partials[3]
    out_flat[1::2] = partials[4] + partials[5] + partials[6] + partials[7]
    # V-bias passes through the normalized attention additively (exact up to
    # the eps clip): attn(v + bv) = attn(v) + bv, so fold bv @ Wo.T into bo.
    bo_eff = np.asarray(bo, np.float32) + np.asarray(bv, np.float32) @ np.asarray(
        Wo, np.float32
    ).T.astype(np.float32)
    out_flat += bo_eff[None, :]
    return out_flat.reshape(BSZ, L, E)


def run(inputs, trace=False):
    from concourse.bass_utils import run_bass_kernel_spmd

    in_maps = make_in_maps(**inputs)
    nc = _get_nc()
    res = run_bass_kernel_spmd(nc, in_maps, list(range(N_CORES)), trace=trace)
    partials = [r["out"] for r in res.results]
    return assemble(partials, inputs["bo"], inputs["bv"], inputs["Wo"]), res


def kernel(**inputs):
    out, _ = run(inputs, trace=False)
    return out


# revision 34
# speedup vs baseline: 1.0529x; 1.0529x over previous
"""Cosformer attention (causal linear attention with cos reweighting) on 8
Trainium2 NeuronCores.

Sharding: n = bsz*heads = 16 sequences -> 2 per core. Core c handles batch-half
i = c//4 and head-pair p = c%4 (heads 2p, 2p+1). Fully data/head parallel; the
only cross-core interaction is the host-side sum of output-projection partials.

Per-core kernel (L=1024 tokens, d=64 per head, pair feature dim P=128):
  1. Feat-major projections Q^T/K^T per head in TF32 (duplicated-W trick:
     weight slice [Wh.T | Wh.T] (512x128) so rows 0:64 / 64:128 both hold the
     head's features) -> relu(+bias) -> * [sin;cos] row table -> bf16 q_^T,k_^T.
     V^T projected once per pair.
  2. Chunked causal linear attention (bf16 matmuls, fp32 PSUM), chunk=128:
       B   = masked A^T (upper-tri j<=i)
       qkv = B.T @ V~  +  q^T.T @ S      (V~ = [V|1]; col 64 = denominator)
       S  += K_tok.T @ V~ in a persistent PSUM bank (fp32, no drift)
       attn = qkv[:,0:64] * 1/max(denom,eps)   (per-partition scalars)
  3. attn pair chunk -> PE transpose -> TF32 out-proj partial -> DRAM.
Host sums 4 partials per batch-half, adds bo, reinterleaves rows.

Inputs per core are packed into three DMA images (xt + 2 constant packs) to
minimize DMA trigger count; loads are split across the two HWDGE rings
(sync/SP and scalar/ACT); output partials go out via gpsimd SWDGE.
"""

import os
import sys

import numpy as np

for _p in ("/opt/trn_rl_repo", "/root/.axon_site/_ro/trn_rl_repo"):
    if os.path.isdir(_p) and _p not in sys.path:
        sys.path.insert(0, _p)

N_HEAD = 8
E = 512
L = 1024  # sequence length per batch-half
BSZ = 2
D = 64  # head dim
P = 128  # partition/chunk/pair-feature size
NCHUNK = L // P
EPS = 1e-6
N_CORES = 8
TH = 512  # token-half width for projections (f32r moving max)

# pack layouts (fp32 columns)
# crit1: [biases(4: bq_a,bq_b,bk_a,bk_b) | wq_a (512)] -- small, lands first
_C1_BIAS = 0
_C1_WQA = 4
_C1_COLS = 516
# crit2: [scb f32 (1024) | wq_b | wk_a | wk_b]
_C2_SCB = 0
_C2_W = 1024
_C2_COLS = 2560
# rest: [wv (512) | wo (512) | mask (128) | ident bf16 (64)]
_R_WV = 0
_R_WO = 512
_R_MASK = 1024
_R_IDENT = 1152
_R_COLS = 1216

_CACHE = {}


def _build_bass():
    import concourse.bass as bass
    import concourse.tile as tile
    from concourse import bacc, mybir
    from contextlib import ExitStack

    f32 = mybir.dt.float32
    f32r = mybir.dt.float32r
    bf16 = mybir.dt.bfloat16
    AF = mybir.ActivationFunctionType

    nc = bacc.Bacc("TRN2", target_bir_lowering=False, debug=False)

    xt_d = nc.dram_tensor("xt", [E, L], f32r, kind="ExternalInput")
    c1_d = nc.dram_tensor("c1", [P, _C1_COLS], f32r, kind="ExternalInput")
    c2_d = nc.dram_tensor("c2", [P, _C2_COLS], f32r, kind="ExternalInput")
    rp_d = nc.dram_tensor("rp", [P, _R_COLS], f32r, kind="ExternalInput")
    out_d = nc.dram_tensor("out", [L, E], f32, kind="ExternalOutput")

    with tile.TileContext(nc) as tc:
        with ExitStack() as ctx:
            ep = ctx.enter_context
            cpool = ep(tc.tile_pool(name="const", bufs=1))
            seqp = ep(tc.tile_pool(name="seq", bufs=1))
            ktokp = ep(tc.tile_pool(name="ktok", bufs=8))
            vtp = ep(tc.tile_pool(name="vt", bufs=10))
            bp = ep(tc.tile_pool(name="bsb", bufs=4))
            sp = ep(tc.tile_pool(name="state", bufs=4))
            app = ep(tc.tile_pool(name="apair", bufs=2))
            atp = ep(tc.tile_pool(name="attnT", bufs=2))
            outp = ep(tc.tile_pool(name="outsb", bufs=2))
            rp = ep(tc.tile_pool(name="rcol", bufs=4))
            big_ps = ep(tc.tile_pool(name="bigps", bufs=2, space="PSUM"))
            sq_ps = ep(tc.tile_pool(name="sqps", bufs=2, space="PSUM"))
            acc_ps = ep(tc.tile_pool(name="accps", bufs=2, space="PSUM"))
            s_ps = ep(tc.tile_pool(name="sps", bufs=1, space="PSUM"))

            # ---- loads: two HWDGE rings (sync + scalar) ----
            # sync: crit packs + xt token-half 1; scalar: xt token-half 0 + rest.
            c1_t = cpool.tile([P, _C1_COLS], f32r, name="c1_t")
            nc.sync.dma_start(c1_t[:], c1_d[:, :])
            c2_t = cpool.tile([P, _C2_COLS], f32r, name="c2_t")
            nc.sync.dma_start(c2_t[:], c2_d[:, :])
            xts = []
            for e in range(4):
                t = cpool.tile([P, L], f32r, name=f"xt{e}")
                nc.scalar.dma_start(t[:, 0:TH], xt_d[e * P : (e + 1) * P, 0:TH])
                nc.sync.dma_start(t[:, TH:L], xt_d[e * P : (e + 1) * P, TH:L])
                xts.append(t)
            rp_t = cpool.tile([P, _R_COLS], f32r, name="rp_t")
            nc.scalar.dma_start(rp_t[:], rp_d[:, :])

            wt = {}
            wt["wq_a"] = [
                c1_t[:, _C1_WQA + e * P : _C1_WQA + (e + 1) * P] for e in range(4)
            ]
            for wi, nm in enumerate(("wq_b", "wk_a", "wk_b")):
                wt[nm] = [
                    c2_t[:, _C2_W + wi * 512 + e * P : _C2_W + wi * 512 + (e + 1) * P]
                    for e in range(4)
                ]
            wt["wv"] = [
                rp_t[:, _R_WV + e * P : _R_WV + (e + 1) * P] for e in range(4)
            ]
            wo_t = rp_t[:, _R_WO : _R_WO + 512]
            scb_t = c2_t[:, _C2_SCB : _C2_SCB + 1024].bitcast(f32)
            mask_t = rp_t[:, _R_MASK : _R_MASK + 128].bitcast(f32)
            bt = {
                nm: c1_t[:, _C1_BIAS + i : _C1_BIAS + i + 1].bitcast(f32)
                for i, nm in enumerate(("bq_a", "bq_b", "bk_a", "bk_b"))
            }
            ident_t = rp_t[:, _R_IDENT : _R_IDENT + 64].bitcast(bf16)

            # ---- projections (feat-major, TF32 matmuls, bf16 outputs) ----
            q_seq = {h: seqp.tile([P, L], bf16, name=f"q_{h}") for h in "ab"}
            k_seq = {h: seqp.tile([P, L], bf16, name=f"k_{h}") for h in "ab"}
            v_seq = seqp.tile([P, L], bf16, name="v_pair")

            def project_half(seq, wname, bname, func, outname, do_scale, th):
                ps = big_ps.tile([P, TH], f32, tag="big", name=f"{outname}_ps{th}")
                for e in range(4):
                    nc.tensor.matmul(
                        ps[:],
                        wt[wname][e],
                        xts[e][:, th * TH : (th + 1) * TH],
                        start=(e == 0),
                        stop=(e == 3),
                    )
                sl = seq[:, th * TH : (th + 1) * TH]
                if bname is None:
                    nc.scalar.copy(sl, ps[:])
                else:
                    nc.scalar.activation(sl, ps[:], func, bias=bt[bname])
                if do_scale:
                    nc.vector.tensor_mul(sl, sl, scb_t[:, th * TH : (th + 1) * TH])

            def project_th(th):
                for h in "ab":
                    project_half(q_seq[h], f"wq_{h}", f"bq_{h}", AF.Relu, f"q_{h}", True, th)
                    project_half(k_seq[h], f"wk_{h}", f"bk_{h}", AF.Relu, f"k_{h}", True, th)
                project_half(v_seq, "wv", None, AF.Identity, "v_pair", False, th)

            # ---- attention ----
            # Per-head fp32 running state, each in its own persistent PSUM
            # bank (start=True zeroes a whole 2KB bank region, so groups can
            # never share a bank); bf16 snapshots feed the next chunk's inter.
            s_bank = {
                "a": s_ps.tile([P, D + 1], f32, name="s_bank_a"),
                "b": s_ps.tile([P, D + 1], f32, name="s_bank_b"),
            }
            S_prev = {"a": None, "b": None}
            vtiles = {}
            ktoks = {}

            def attn_chunk(c):
                cs = slice(c * P, (c + 1) * P)
                vt_ps = sq_ps.tile([P, P], bf16, tag="sq", name=f"vtps{c}")
                nc.tensor.matmul(vt_ps[:], v_seq[:, cs], ident_t, is_transpose=True)
                for j, h in enumerate("ab"):
                    vt = vtp.tile([P, D + 1], bf16, tag="vt", name=f"vt_{h}{c}")
                    nc.vector.tensor_copy(vt[:, 0:D], vt_ps[:, j * D : (j + 1) * D])
                    nc.gpsimd.memset(vt[:, D : D + 1], 1.0)
                    vtiles[h, c] = vt
                attn_pair = app.tile([P, P], bf16, tag="ap", name=f"ap{c}")
                for j, h in enumerate("ab"):
                    qc = q_seq[h][:, cs]
                    kc = k_seq[h][:, cs]
                    vt = vtiles[h, c]
                    # masked A^T
                    b_ps = sq_ps.tile([P, P], f32, tag="sq", name=f"bps_{h}{c}")
                    nc.tensor.matmul(b_ps[:], kc, qc, start=True, stop=True)
                    b_sb = bp.tile([P, P], bf16, tag="bsb", name=f"bsb_{h}{c}")
                    nc.vector.tensor_mul(b_sb[:], b_ps[:], mask_t)
                    # qkv = intra + inter
                    qkv = acc_ps.tile([P, D + 1], f32, tag="acc", name=f"qkv_{h}{c}")
                    nc.tensor.matmul(
                        qkv[:], b_sb[:], vt[:], start=True, stop=(c == 0)
                    )
                    if c > 0:
                        nc.tensor.matmul(
                            qkv[:], qc, S_prev[h][:], start=False, stop=True
                        )
                    # state update in persistent PSUM (skip on last chunk)
                    if c < NCHUNK - 1:
                        kt_ps = sq_ps.tile([P, P], bf16, tag="sq", name=f"ktps_{h}{c}")
                        nc.tensor.matmul(
                            kt_ps[:], kc, ident_t, is_transpose=True
                        )
                        ktok = ktokp.tile([P, P], bf16, tag="ktok", name=f"ktok_{h}{c}")
                        nc.vector.tensor_copy(ktok[:], kt_ps[:])
                        nc.tensor.matmul(
                            s_bank[h][:],
                            ktok[:],
                            vt[:],
                            start=(c == 0),
                            stop=(c == NCHUNK - 2),
                            skip_group_check=True,
                        )
                        s_new = sp.tile([P, D + 1], bf16, tag="S", name=f"S_{h}{c}")
                        nc.scalar.copy(s_new[:], s_bank[h][:])
                        S_prev[h] = s_new
                    # normalize
                    r_col = rp.tile([P, 2], f32, tag="r", name=f"r_{h}{c}")
                    nc.vector.tensor_scalar_max(r_col[:, 0:1], qkv[:, D : D + 1], EPS)
                    nc.vector.reciprocal(r_col[:, 1:2], r_col[:, 0:1])
                    nc.vector.tensor_scalar_mul(
                        attn_pair[:, j * D : (j + 1) * D], qkv[:, 0:D], r_col[:, 1:2]
                    )
                # out projection for this chunk (TF32)
                at_ps = acc_ps.tile([P, P], bf16, tag="acc", name=f"atps{c}")
                nc.tensor.matmul(at_ps[:], attn_pair[:], ident_t, is_transpose=True)
                at_sb = atp.tile([P, P], f32r, tag="at", name=f"at{c}")
                nc.scalar.copy(at_sb[:], at_ps[:])
                o_ps = big_ps.tile([P, E], f32, tag="big", name=f"ops{c}")
                nc.tensor.matmul(o_ps[:], at_sb[:], wo_t, start=True, stop=True)
                o_sb = outp.tile([P, E], f32, tag="osb", name=f"osb{c}")
                if c % 2 == 0:
                    nc.scalar.copy(o_sb[:], o_ps[:])
                else:
                    nc.vector.tensor_copy(o_sb[:], o_ps[:])
                nc.gpsimd.dma_start(out_d[cs, :], o_sb[:])

            project_th(0)
            project_th(1)
            for c in range(NCHUNK):
                attn_chunk(c)

    nc.compile()
    return nc


def _get_nc():
    if "nc" not in _CACHE:
        _CACHE["nc"] = _build_bass()
    return _CACHE["nc"]


def make_in_maps(query, Wq, bq, Wk, bk, Wv, bv, Wo, bo):
    import ml_dtypes

    f32 = np.float32
    query = np.asarray(query, f32)
    x3 = query.reshape(L, BSZ, E)  # faithful torch .view reshape
    idx = (np.pi / 2) * np.arange(1, L + 1, dtype=f32) / f32(L)
    sinv = np.sin(idx).astype(f32)
    cosv = np.cos(idx).astype(f32)

    Wq, Wk, Wv, Wo = (np.asarray(w, f32) for w in (Wq, Wk, Wv, Wo))
    bq, bk, bv = (np.asarray(b, f32) for b in (bq, bk, bv))

    def wslice_dup(W, h):
        """(128, 512): [Wh.T | Wh.T] dup cols laid out as 4 e-tiles of 128."""
        w = W[D * h : D * (h + 1), :].T  # (512, 64)
        wd = np.concatenate([w, w], axis=1)  # (512, 128)
        return np.hstack([wd[e * P : (e + 1) * P, :] for e in range(4)])

    def bdup(b, h):
        bb = b[D * h : D * (h + 1)]
        return np.concatenate([bb, bb]).astype(f32)

    ident_f32 = np.ascontiguousarray(np.eye(P, dtype=ml_dtypes.bfloat16)).view(f32)
    scb = np.empty((P, L), f32)
    scb[0:D] = sinv[None, :]
    scb[D:P] = cosv[None, :]
    mask = np.triu(np.ones((P, P), f32))

    in_maps = []
    for c in range(N_CORES):
        i, p = divmod(c, 4)
        hA, hB = 2 * p, 2 * p + 1

        biases = np.stack(
            [bdup(bq, hA), bdup(bq, hB), bdup(bk, hA), bdup(bk, hB)], axis=1
        ).astype(f32)  # (128, 4)
        c1 = np.hstack([biases, wslice_dup(Wq, hA)])
        assert c1.shape == (P, _C1_COLS), c1.shape
        c2 = np.hstack(
            [scb, wslice_dup(Wq, hB), wslice_dup(Wk, hA), wslice_dup(Wk, hB)]
        )
        assert c2.shape == (P, _C2_COLS), c2.shape

        wv_p = Wv[P * p : P * (p + 1), :].T  # (512, 128)
        wv_pack = np.hstack([wv_p[e * P : (e + 1) * P, :] for e in range(4)])
        wo_pack = Wo[:, P * p : P * (p + 1)].T  # (128, 512)
        rest = np.hstack([wv_pack, wo_pack, mask, ident_f32])
        assert rest.shape == (P, _R_COLS), rest.shape

        in_maps.append(
            dict(
                xt=np.ascontiguousarray(x3[:, i, :].T),
                c1=np.ascontiguousarray(c1),
                c2=np.ascontiguousarray(c2),
                rp=np.ascontiguousarray(rest),
            )
        )
    return in_maps


def assemble(partials, bo, bv, Wo):
    out_flat = np.zeros((BSZ * L, E), np.float32)
    out_flat[0::2] = partials[0] + partials[1] + partials[2] + partials[3]
    out_flat[1::2] = partials[4] + partials[5] + partials[6] + partials[7]
    # V-bias passes through the normalized attention additively (exact up to
    # the eps clip): attn(v + bv) = attn(v) + bv, so fold bv @ Wo.T into bo.
    bo_eff = np.asarray(bo, np.float32) + np.asarray(bv, np.float32) @ np.asarray(
        Wo, np.float32
    ).T.astype(np.float32)
    out_flat += bo_eff[None, :]
    return out_flat.reshape(BSZ, L, E)


def run(inputs, trace=False):
    from concourse.bass_utils import run_bass_kernel_spmd

    in_maps = make_in_maps(**inputs)
    nc = _get_nc()
    res = run_bass_kernel_spmd(nc, in_maps, list(range(N_CORES)), trace=trace)
    partials = [r["out"] for r in res.results]
    return assemble(partials, inputs["bo"], inputs["bv"], inputs["Wo"]), res


def kernel(**inputs):
    out, _ = run(inputs, trace=False)
    return out
